# revision 5
# baseline (speedup 1.0000x reference)
"""Mixtral-style MoE (E=8, top-2, H=1024, F=3584, T=2048) on 8 TRN2 NeuronCores.

Strategy: expert-parallel. Host computes the (tiny) router, gathers each
expert's assigned tokens (the MoE all-to-all dispatch done as input sharding),
each core runs a 3-matmul SiLU-gated MLP for ONE expert over only its routed
tokens (~4x FLOP cut vs the dense reference) in bf16, and the host
scatter-adds the 8 weighted partial outputs (the all-reduce combine done as
output unsharding).

Per-core kernel layout (all matmuls out = lhsT.T @ rhs, contraction on
partitions):
  phase 1: for each F-tile f (28 of 128):  gT/uT [128f, C] = sum_k
           w1T[k,f].T @ xT[k, :]  (k = 8 H-chunks of 128), PSUM-accumulated;
           actT[:, f, :] = bf16(silu(gT) * uT)         (ACT + DVE)
  phase 2: for each token tile t (128):  y [128t, 1024] = sum_f
           actT[:, f, t].T @ w2T[f, :]; y *= combine_weight[token] (DVE),
           DMA out.
"""

import numpy as np
import ml_dtypes

import concourse.bass as bass
import concourse.mybir as mybir
import concourse.tile as tile_mod
from concourse.tile import TileContext
from concourse.vector_clock import ScopedClock, VectorClock
from concourse.bass_utils import run_bass_kernel_spmd

E, K, H, F = 8, 2, 1024, 3584
NCORES = 8
BF16 = mybir.dt.bfloat16
F32 = mybir.dt.float32
NPBF16 = ml_dtypes.bfloat16


def _patched_drain_and_barrier(self, tick_clock, wait_clock):
    # The stock TileContext exit stacks every outstanding proc's sem wait on
    # one Drain instruction; this walrus build rejects >1 sync wait there
    # ("Too many sync wait commands"). Emit one single-wait NOP per proc on
    # the sync engine instead, then a clean drain.
    gc = tick_clock.global_clock
    n = len(gc)
    for p in range(n):
        if gc[p] > 0:
            vc = VectorClock([gc[q] if q == p else 0 for q in range(n)])
            w = self.nc.sync.nop(nofuse=True, hint="tile_exit_wait")
            wait_clock.add_sem_waits(w.ins, ScopedClock({None: vc}))
    self.nc.sync.drain()
    self.nc.all_engine_barrier()
    popped = self.nc._tile_sem_poison_stack.pop()
    assert popped is self._sem_poison
    self.nc.clear_and_free_semaphores(list(self.sems.allocated().values()))
    self.nc.all_engine_barrier()


tile_mod.TileContext._drain_and_barrier = _patched_drain_and_barrier


def _split_multi_waits(bir_json: bytes) -> bytes:
    """This walrus build rejects instructions carrying multiple sync waits.
    Hoist all-but-one wait of every instruction onto single-wait NoOps
    inserted immediately before it on the same engine (semantically identical:
    sem waits are monotonic and NX executes the stream in order)."""
    import json as _json

    bir = _json.loads(bir_json)
    ctr = 0
    for fn in bir.get("functions", []):
        for blk in fn.get("blocks", []):
            out = []
            for ins in blk.get("instructions", []):
                si = ins.get("sync_info") or {}
                w = si.get("on_wait") or []
                if len(w) > 1:
                    for extra in w[:-1]:
                        ctr += 1
                        out.append({
                            "debug": ins.get("debug", 0),
                            "engine": ins["engine"],
                            "ins": [],
                            "outs": [],
                            "name": f"I-waitsplit-{ctr}",
                            "opcode": "NoOp",
                            "sync_info": {"on_update": [], "on_wait": [extra]},
                        })
                    si["on_wait"] = [w[-1]]
                out.append(ins)
            blk["instructions"] = out
    return _json.dumps(bir).encode()


import concourse.bass_utils as _bass_utils_mod
import concourse.bass2jax as _bass2jax_mod

_orig_compile_bir_kernel = _bass_utils_mod.compile_bir_kernel


def _patched_compile_bir_kernel(bir_json, tmpdir, neff_name="file.neff"):
    return _orig_compile_bir_kernel(_split_multi_waits(bir_json), tmpdir,
                                    neff_name=neff_name)


_bass_utils_mod.compile_bir_kernel = _patched_compile_bir_kernel
_bass2jax_mod.compile_bir_kernel = _patched_compile_bir_kernel

# If BASS_TRACE is set but this container lacks the axon NTFF hook module,
# run_bass_kernel_spmd would crash on import. Stub it to "hook unavailable"
# so tracing degrades gracefully; a real hook, when present, is untouched.
try:
    import antenv.axon_hooks  # noqa: F401
except ImportError:
    import sys as _sys
    import types as _types
    import antenv as _antenv

    _stub = _types.ModuleType("antenv.axon_hooks")
    _stub.get_axon_ntff_profile_hook = lambda: None
    _sys.modules["antenv.axon_hooks"] = _stub
    _antenv.axon_hooks = _stub


def _route(x, gate_w):
    """Replicate the reference router in numpy fp32."""
    logits = x @ gate_w.T                                   # [T, E] f32
    m = logits.max(axis=-1, keepdims=True)
    e = np.exp(logits - m, dtype=np.float32)
    rw = e / e.sum(axis=-1, keepdims=True)                  # softmax [T, E]
    topk_idx = np.argsort(-rw, axis=-1, kind="stable")[:, :K]  # [T, K]
    topk_w = np.take_along_axis(rw, topk_idx, axis=-1)
    topk_w = topk_w / topk_w.sum(axis=-1, keepdims=True)
    return topk_idx.astype(np.int64), topk_w.astype(np.float32)


def _ceil_to(v, m):
    return -(-v // m) * m


def _build_bass(C):
    """Per-core Tile kernel at token capacity C (multiple of 64)."""
    KH = H // 128          # 8 H-chunks
    NF = F // 128          # 28 F-tiles
    NT = -(-C // 128)      # token tiles (last may be partial)
    c_chunks = []
    off = 0
    while off < C:
        w = min(512, C - off)
        c_chunks.append((off, w))
        off += w

    nc = bass.Bass()
    xt_d = nc.dram_tensor("xt", [KH, 128, C], BF16, kind="ExternalInput")
    w1t_d = nc.dram_tensor("w1t", [NF, 128, KH, 128], BF16, kind="ExternalInput")
    w3t_d = nc.dram_tensor("w3t", [NF, 128, KH, 128], BF16, kind="ExternalInput")
    w2t_d = nc.dram_tensor("w2t", [NF, 128, H], BF16, kind="ExternalInput")
    wv_d = nc.dram_tensor("wv", [128, NT], F32, kind="ExternalInput")
    y_d = nc.dram_tensor("y", [C, H], F32, kind="ExternalOutput")

    with TileContext(nc) as tc:
        with (
            tc.tile_pool(name="resident", bufs=1) as res,
            tc.tile_pool(name="wstream", bufs=3) as wstream,
            tc.tile_pool(name="tmp", bufs=2) as tmp,
            tc.tile_pool(name="psum", bufs=2, space="PSUM") as psum,
        ):
            xt_sb = res.tile([128, KH, C], BF16, tag="xt")
            act_sb = res.tile([128, NF, C], BF16, tag="act")
            w2_sb = res.tile([128, NF, H], BF16, tag="w2")
            wv_sb = res.tile([128, NT], F32, tag="wv")

            nc.sync.dma_start(wv_sb[:], wv_d[:])
            for k in range(KH):
                nc.sync.dma_start(xt_sb[:, k, :], xt_d[k])
            for f in range(NF):
                nc.sync.dma_start(w2_sb[:, f, :], w2t_d[f])

            # ---- phase 1: gT/uT = w1/w3 contractions over H; act = silu(g)*u
            for f in range(NF):
                w1_sb = wstream.tile([128, KH, 128], BF16, tag="w1")
                w3_sb = wstream.tile([128, KH, 128], BF16, tag="w3")
                nc.sync.dma_start(w1_sb[:], w1t_d[f])
                nc.sync.dma_start(w3_sb[:], w3t_d[f])
                for ci, (c0, cw) in enumerate(c_chunks):
                    g_ps = psum.tile([128, cw], F32, tag=f"g{ci}")
                    u_ps = psum.tile([128, cw], F32, tag=f"u{ci}")
                    for k in range(KH):
                        nc.tensor.matmul(
                            g_ps[:], w1_sb[:, k, :], xt_sb[:, k, c0:c0 + cw],
                            start=(k == 0), stop=(k == KH - 1),
                        )
                    for k in range(KH):
                        nc.tensor.matmul(
                            u_ps[:], w3_sb[:, k, :], xt_sb[:, k, c0:c0 + cw],
                            start=(k == 0), stop=(k == KH - 1),
                        )
                    s_sb = tmp.tile([128, cw], F32, tag=f"silu{ci}")
                    nc.scalar.activation(
                        s_sb[:], g_ps[:], mybir.ActivationFunctionType.Silu
                    )
                    nc.vector.tensor_tensor(
                        act_sb[:, f, c0:c0 + cw], s_sb[:], u_ps[:],
                        mybir.AluOpType.mult,
                    )

            # ---- phase 2: y = actT.T @ w2T, scaled by combine weights
            for t in range(NT):
                t0 = t * 128
                pt = min(128, C - t0)
                y_sb = tmp.tile([128, H], F32, tag="y")
                for hh in range(H // 512):
                    y_ps = psum.tile([128, 512], F32, tag=("g0" if hh % 2 == 0 else "u0"))
                    for f in range(NF):
                        nc.tensor.matmul(
                            y_ps[:pt, :], act_sb[:, f, t0:t0 + pt],
                            w2_sb[:, f, hh * 512:(hh + 1) * 512],
                            start=(f == 0), stop=(f == NF - 1),
                        )
                    nc.vector.tensor_scalar_mul(
                        y_sb[:pt, hh * 512:(hh + 1) * 512], y_ps[:pt, :],
                        wv_sb[:pt, t:t + 1],
                    )
                nc.sync.dma_start(y_d[t0:t0 + pt, :], y_sb[:pt, :])

    return nc


def kernel(hidden_states, gate_w, w1, w3, w2):
    x = np.ascontiguousarray(np.asarray(hidden_states, np.float32)).reshape(-1, H)
    gate_w = np.asarray(gate_w, np.float32)
    w1 = np.asarray(w1, np.float32)
    w3 = np.asarray(w3, np.float32)
    w2 = np.asarray(w2, np.float32)
    T = x.shape[0]

    topk_idx, topk_w = _route(x, gate_w)

    idx_e, wv_e = [], []
    for e in range(E):
        sel_t, sel_k = np.nonzero(topk_idx == e)
        idx_e.append(sel_t)
        wv_e.append(topk_w[sel_t, sel_k])
    maxT = max(len(i) for i in idx_e)
    C = max(128, _ceil_to(maxT, 64))
    NT = -(-C // 128)

    xbf = x.astype(NPBF16)
    in_maps = []
    for e in range(E):
        n = len(idx_e[e])
        xg = np.zeros((C, H), NPBF16)
        xg[:n] = xbf[idx_e[e]]
        xt = np.ascontiguousarray(xg.T).reshape(H // 128, 128, C)
        w1t = np.ascontiguousarray(
            w1[e].astype(NPBF16).reshape(F // 128, 128, H // 128, 128)
            .transpose(0, 3, 2, 1)
        )
        w3t = np.ascontiguousarray(
            w3[e].astype(NPBF16).reshape(F // 128, 128, H // 128, 128)
            .transpose(0, 3, 2, 1)
        )
        w2t = np.ascontiguousarray(w2[e].T.astype(NPBF16)).reshape(F // 128, 128, H)
        wv = np.zeros(NT * 128, np.float32)
        wv[:n] = wv_e[e]
        wv = np.ascontiguousarray(wv.reshape(NT, 128).T)[:, :NT]
        in_maps.append({"xt": xt, "w1t": w1t, "w3t": w3t, "w2t": w2t, "wv": wv})

    nc = _build_bass(C)
    res = run_bass_kernel_spmd(nc, in_maps, core_ids=list(range(NCORES)))
    global last_results, last_in_maps, last_C
    last_results, last_in_maps, last_C = res, in_maps, C

    out = np.zeros((T, H), np.float32)
    for e in range(E):
        n = len(idx_e[e])
        out[idx_e[e]] += res.results[e]["y"][:n]
    return out.reshape(1, T, H).astype(np.float32)


# revision 8
# speedup vs baseline: 5.4715x; 5.4715x over previous
"""Mixtral-style MoE (E=8, top-2, H=1024, F=3584, T=2048) on 8 TRN2 NeuronCores.

Strategy: expert-parallel. Host computes the (tiny) router, gathers each
expert's assigned tokens (the MoE all-to-all dispatch done as input sharding),
each core runs a 3-matmul SiLU-gated MLP for ONE expert over only its routed
tokens (~4x FLOP cut vs the dense reference) in bf16, and the host
scatter-adds the 8 weighted partial outputs (the all-reduce combine done as
output unsharding).

Per-core kernel layout (all matmuls out = lhsT.T @ rhs, contraction on
partitions):
  phase 1: for each F-tile f (28 of 128):  gT/uT [128f, C] = sum_k
           w1T[k,f].T @ xT[k, :]  (k = 8 H-chunks of 128), PSUM-accumulated;
           actT[:, f, :] = bf16(silu(gT) * uT)         (ACT + DVE)
  phase 2: for each token tile t (128):  y [128t, 1024] = sum_f
           actT[:, f, t].T @ w2T[f, :]; y *= combine_weight[token] (DVE),
           DMA out.
"""

import numpy as np
import ml_dtypes

import concourse.bass as bass
import concourse.mybir as mybir
import concourse.tile as tile_mod
from concourse.tile import TileContext
from concourse.vector_clock import ScopedClock, VectorClock
from concourse.bass_utils import run_bass_kernel_spmd

E, K, H, F = 8, 2, 1024, 3584
NCORES = 8
BF16 = mybir.dt.bfloat16
F32 = mybir.dt.float32
NPBF16 = ml_dtypes.bfloat16


def _patched_drain_and_barrier(self, tick_clock, wait_clock):
    # The stock TileContext exit stacks every outstanding proc's sem wait on
    # one Drain instruction; this walrus build rejects >1 sync wait there
    # ("Too many sync wait commands"). Emit one single-wait NOP per proc on
    # the sync engine instead, then a clean drain.
    gc = tick_clock.global_clock
    n = len(gc)
    for p in range(n):
        if gc[p] > 0:
            vc = VectorClock([gc[q] if q == p else 0 for q in range(n)])
            w = self.nc.sync.nop(nofuse=True, hint="tile_exit_wait")
            wait_clock.add_sem_waits(w.ins, ScopedClock({None: vc}))
    self.nc.sync.drain()
    self.nc.all_engine_barrier()
    popped = self.nc._tile_sem_poison_stack.pop()
    assert popped is self._sem_poison
    self.nc.clear_and_free_semaphores(list(self.sems.allocated().values()))
    self.nc.all_engine_barrier()


tile_mod.TileContext._drain_and_barrier = _patched_drain_and_barrier


def _split_multi_waits(bir_json: bytes) -> bytes:
    """This walrus build rejects instructions carrying multiple sync waits.
    Hoist all-but-one wait of every instruction onto single-wait NoOps
    inserted immediately before it on the same engine (semantically identical:
    sem waits are monotonic and NX executes the stream in order)."""
    import json as _json

    bir = _json.loads(bir_json)
    ctr = 0
    for fn in bir.get("functions", []):
        for blk in fn.get("blocks", []):
            out = []
            for ins in blk.get("instructions", []):
                si = ins.get("sync_info") or {}
                w = si.get("on_wait") or []
                if len(w) > 1:
                    for extra in w[:-1]:
                        ctr += 1
                        out.append({
                            "debug": ins.get("debug", 0),
                            "engine": ins["engine"],
                            "ins": [],
                            "outs": [],
                            "name": f"I-waitsplit-{ctr}",
                            "opcode": "NoOp",
                            "sync_info": {"on_update": [], "on_wait": [extra]},
                        })
                    si["on_wait"] = [w[-1]]
                out.append(ins)
            blk["instructions"] = out
    return _json.dumps(bir).encode()


import concourse.bass_utils as _bass_utils_mod
import concourse.bass2jax as _bass2jax_mod

_orig_compile_bir_kernel = _bass_utils_mod.compile_bir_kernel


def _patched_compile_bir_kernel(bir_json, tmpdir, neff_name="file.neff"):
    return _orig_compile_bir_kernel(_split_multi_waits(bir_json), tmpdir,
                                    neff_name=neff_name)


_bass_utils_mod.compile_bir_kernel = _patched_compile_bir_kernel
_bass2jax_mod.compile_bir_kernel = _patched_compile_bir_kernel

# If BASS_TRACE is set but this container lacks the axon NTFF hook module,
# run_bass_kernel_spmd would crash on import. Stub it to "hook unavailable"
# so tracing degrades gracefully; a real hook, when present, is untouched.
try:
    import antenv.axon_hooks  # noqa: F401
except ImportError:
    import sys as _sys
    import types as _types
    import antenv as _antenv

    _stub = _types.ModuleType("antenv.axon_hooks")
    _stub.get_axon_ntff_profile_hook = lambda: None
    _sys.modules["antenv.axon_hooks"] = _stub
    _antenv.axon_hooks = _stub


def _route(x, gate_w):
    """Replicate the reference router in numpy fp32."""
    logits = x @ gate_w.T                                   # [T, E] f32
    m = logits.max(axis=-1, keepdims=True)
    e = np.exp(logits - m, dtype=np.float32)
    rw = e / e.sum(axis=-1, keepdims=True)                  # softmax [T, E]
    topk_idx = np.argsort(-rw, axis=-1, kind="stable")[:, :K]  # [T, K]
    topk_w = np.take_along_axis(rw, topk_idx, axis=-1)
    topk_w = topk_w / topk_w.sum(axis=-1, keepdims=True)
    return topk_idx.astype(np.int64), topk_w.astype(np.float32)


def _ceil_to(v, m):
    return -(-v // m) * m


def _build_bass(C):
    """Per-core Tile kernel at token capacity C (multiple of 64)."""
    KH = H // 128          # 8 H-chunks
    NF = F // 128          # 28 F-tiles
    NT = -(-C // 128)      # token tiles (last may be partial)
    c_chunks = []
    off = 0
    while off < C:
        w = min(512, C - off)
        c_chunks.append((off, w))
        off += w

    nc = bass.Bass()
    xt_d = nc.dram_tensor("xt", [KH, 128, C], BF16, kind="ExternalInput")
    w1t_d = nc.dram_tensor("w1t", [NF, 128, KH, 128], BF16, kind="ExternalInput")
    w3t_d = nc.dram_tensor("w3t", [NF, 128, KH, 128], BF16, kind="ExternalInput")
    w2t_d = nc.dram_tensor("w2t", [NF, 128, H], BF16, kind="ExternalInput")
    wv_d = nc.dram_tensor("wv", [128, NT], F32, kind="ExternalInput")
    y_d = nc.dram_tensor("y", [C, H], F32, kind="ExternalOutput")

    with TileContext(nc) as tc:
        with (
            tc.tile_pool(name="resident", bufs=1) as res,
            tc.tile_pool(name="wstream", bufs=3) as wstream,
            tc.tile_pool(name="tmp", bufs=2) as tmp,
            tc.tile_pool(name="psum", bufs=2, space="PSUM") as psum,
        ):
            xt_sb = res.tile([128, KH, C], BF16, tag="xt")
            act_sb = res.tile([128, NF, C], BF16, tag="act")
            w2_sb = res.tile([128, NF, H], BF16, tag="w2")
            wv_sb = res.tile([128, NT], F32, tag="wv")

            # Spread bulk loads across both HWDGE rings (SP + ACT) and the
            # SWDGE (gpsimd) so the ~23MB of weights doesn't serialize on one
            # DMA ring.
            dma_engines = [nc.sync, nc.scalar, nc.gpsimd]
            nc.sync.dma_start(wv_sb[:], wv_d[:])
            for k in range(KH):
                dma_engines[k % 3].dma_start(xt_sb[:, k, :], xt_d[k])
            for f in range(NF):
                dma_engines[f % 3].dma_start(w2_sb[:, f, :], w2t_d[f])

            # ---- phase 1: gT/uT = w1/w3 contractions over H; act = silu(g)*u
            for f in range(NF):
                w1_sb = wstream.tile([128, KH, 128], BF16, tag="w1")
                w3_sb = wstream.tile([128, KH, 128], BF16, tag="w3")
                dma_engines[f % 3].dma_start(w1_sb[:], w1t_d[f])
                dma_engines[(f + 1) % 3].dma_start(w3_sb[:], w3t_d[f])
                for ci, (c0, cw) in enumerate(c_chunks):
                    g_ps = psum.tile([128, cw], F32, tag=f"g{ci}")
                    u_ps = psum.tile([128, cw], F32, tag=f"u{ci}")
                    for k in range(KH):
                        nc.tensor.matmul(
                            g_ps[:], w1_sb[:, k, :], xt_sb[:, k, c0:c0 + cw],
                            start=(k == 0), stop=(k == KH - 1),
                        )
                    for k in range(KH):
                        nc.tensor.matmul(
                            u_ps[:], w3_sb[:, k, :], xt_sb[:, k, c0:c0 + cw],
                            start=(k == 0), stop=(k == KH - 1),
                        )
                    s_sb = tmp.tile([128, cw], F32, tag=f"silu{ci}")
                    nc.scalar.activation(
                        s_sb[:], g_ps[:], mybir.ActivationFunctionType.Silu
                    )
                    nc.vector.tensor_tensor(
                        act_sb[:, f, c0:c0 + cw], s_sb[:], u_ps[:],
                        mybir.AluOpType.mult,
                    )

            # ---- phase 2: y = actT.T @ w2T, scaled by combine weights
            for t in range(NT):
                t0 = t * 128
                pt = min(128, C - t0)
                y_sb = tmp.tile([128, H], F32, tag="y")
                for hh in range(H // 512):
                    y_ps = psum.tile([128, 512], F32, tag=("g0" if hh % 2 == 0 else "u0"))
                    for f in range(NF):
                        nc.tensor.matmul(
                            y_ps[:pt, :], act_sb[:, f, t0:t0 + pt],
                            w2_sb[:, f, hh * 512:(hh + 1) * 512],
                            start=(f == 0), stop=(f == NF - 1),
                        )
                    nc.vector.tensor_scalar_mul(
                        y_sb[:pt, hh * 512:(hh + 1) * 512], y_ps[:pt, :],
                        wv_sb[:pt, t:t + 1],
                    )
                nc.sync.dma_start(y_d[t0:t0 + pt, :], y_sb[:pt, :])

    return nc


def kernel(hidden_states, gate_w, w1, w3, w2):
    x = np.ascontiguousarray(np.asarray(hidden_states, np.float32)).reshape(-1, H)
    gate_w = np.asarray(gate_w, np.float32)
    w1 = np.asarray(w1, np.float32)
    w3 = np.asarray(w3, np.float32)
    w2 = np.asarray(w2, np.float32)
    T = x.shape[0]

    topk_idx, topk_w = _route(x, gate_w)

    idx_e, wv_e = [], []
    for e in range(E):
        sel_t, sel_k = np.nonzero(topk_idx == e)
        idx_e.append(sel_t)
        wv_e.append(topk_w[sel_t, sel_k])
    maxT = max(len(i) for i in idx_e)
    C = max(128, _ceil_to(maxT, 64))
    NT = -(-C // 128)

    xbf = x.astype(NPBF16)
    in_maps = []
    for e in range(E):
        n = len(idx_e[e])
        xg = np.zeros((C, H), NPBF16)
        xg[:n] = xbf[idx_e[e]]
        xt = np.ascontiguousarray(xg.T).reshape(H // 128, 128, C)
        w1t = np.ascontiguousarray(
            w1[e].astype(NPBF16).reshape(F // 128, 128, H // 128, 128)
            .transpose(0, 3, 2, 1)
        )
        w3t = np.ascontiguousarray(
            w3[e].astype(NPBF16).reshape(F // 128, 128, H // 128, 128)
            .transpose(0, 3, 2, 1)
        )
        w2t = np.ascontiguousarray(w2[e].T.astype(NPBF16)).reshape(F // 128, 128, H)
        wv = np.zeros(NT * 128, np.float32)
        wv[:n] = wv_e[e]
        wv = np.ascontiguousarray(wv.reshape(NT, 128).T)[:, :NT]
        in_maps.append({"xt": xt, "w1t": w1t, "w3t": w3t, "w2t": w2t, "wv": wv})

    nc = _build_bass(C)
    res = run_bass_kernel_spmd(nc, in_maps, core_ids=list(range(NCORES)))
    global last_results, last_in_maps, last_C
    last_results, last_in_maps, last_C = res, in_maps, C

    out = np.zeros((T, H), np.float32)
    for e in range(E):
        n = len(idx_e[e])
        out[idx_e[e]] += res.results[e]["y"][:n]
    return out.reshape(1, T, H).astype(np.float32)
